# revision 2
# baseline (speedup 1.0000x reference)
"""MoE-SIREN (nn_MoE_36146444763329) Trainium2 Bass kernel, v2: table+interp.

Each expert's network is a univariate function y_e(x), x in [0,1]. Instead of
evaluating the 4-layer SIREN at every point (33.5M sins/core), each core:
  1. builds, ON DEVICE, a sampled table of all 8 experts over ITS x-range
     (range-sharded points): 72-sample batch through the network in fp16
     (fp32 PSUM accumulate, fp32 range-reduction, ACT Sin),
  2. converts the table to per-bin Catmull-Rom cubic coefficients,
  3. evaluates the cubic + softmax gate at its points in fp32.

Points are range-sharded on the host: core j gets the points with
x in [j/8,(j+1)/8), laid out in a [bin x slot] padded grid so the per-bin
coefficients become per-partition scalars (no gather needed). Host binning is
index computation only; all value arithmetic runs on device. Cubic-vs-exact
error is ~7e-6 (fp32) / ~1.6e-3 (fp16 weights), vs the 2e-2 gate.

Layout per core: M=512 global bins, 64 bins/core, C slots/bin (C>=112,
grown if an input ever overflows; seed-0 max occupancy is 86).
Eval rows: partition p = e*16 + bl (expert e, bin-low bl), 4 tiles of
[128, C] covering bins tt*16+bl. Weighted combine via a [128,16] 0/1
group-indicator matmul that sums the 8 expert rows of each bin.
"""

import numpy as np

import concourse.bass as bass
import concourse.mybir as mybir
import concourse.tile as tile
from concourse import bacc
from concourse.bass_utils import run_bass_kernel_spmd
from concourse.dve_ops import ADD_RANGE_WRAP
from concourse.tile_rust import add_dep_helper

F32 = mybir.dt.float32
F16 = mybir.dt.float16
F32R = mybir.dt.float32r
AT = mybir.ActivationFunctionType
ALU = mybir.AluOpType

B, N, E, H, NLAYERS = 2, 16384, 8, 256, 4
OMEGA0 = 30.0
NCORES = 8
NHID = NLAYERS - 1
TWO_PI = float(2.0 * np.pi)
SC = float(OMEGA0 / (2.0 * np.pi))
MAGIC = float(np.float32(1.5 * 2 ** 23))

M = 512                      # global bins
BPC = M // NCORES            # 64 bins per core
NTILE = 4                    # eval partition-tiles per core (4*128 rows)
NS = 68                      # build sample batch (67 used: bins+3, padded)
# PSUM bank = 512 fp32/partition and a matmul output may not cross a bank
# boundary: place 7 NS-wide units per 512-col bank (pad 8 cols), 16 units
# -> 3 banks. Pad columns hold garbage; they are never read by matmuls.
SLAB = 3 * 512               # build slab width incl. bank padding


def ucol(u):
    return 512 * (u // 7) + NS * (u % 7)


# elementwise spans covering exactly the written unit regions (pads excluded).
# The first span holds only units 0,1 (= expert 0's rhs blocks) so the next
# layer's first matmuls unblock after a small wrap+sin, not a 476-col one.
SPANS = [(0, 2 * NS), (2 * NS, 7 * NS), (512, 512 + 7 * NS),
         (1024, 1024 + 2 * NS)]

# consts tile [128, 64] column layout
C_A0 = 0      # 16: layer-0 scale per unit u=e*2+half
C_C0 = 16     # 16: layer-0 bias
C_GW = 32     # 1: gate weight per row (e*16+bl)
C_GB = 33     # 1: gate bias per row
C_BIN = 34    # 4: global bin index per row, per tile
C_BO = 38     # 1: output bias (partitions 0..7)

_BUILD_CACHE: dict = {}
LAST_RESULT = None


def _build(wrap_twice: bool, C: int):
    nc = bacc.Bacc("TRN2", target_bir_lowering=False, debug=False,
                   num_devices=NCORES)

    # merged inputs: cf32 = cst(64) | xs(NS) | xb(NTILE*C)
    CW = 64 + NS + NTILE * C
    d_g = nc.dram_tensor("g", [128, 64], F32, kind="ExternalInput")
    d_cf32 = nc.dram_tensor("cf32", [128, CW], F32, kind="ExternalInput")
    d_r16 = nc.dram_tensor("r16", [1, NHID * 16 * 128 + NS], F16,
                           kind="ExternalInput")
    d_wh = nc.dram_tensor("wh", [128, NHID * 4096 + 128], F16,
                          kind="ExternalInput")
    d_out = nc.dram_tensor("out", [64, NTILE * C], F32, kind="ExternalOutput")

    with tile.TileContext(nc) as tc:
        with (
            tc.tile_pool(name="cst", bufs=1) as cst_pool,
            tc.tile_pool(name="whp", bufs=1) as wh_pool,
            tc.tile_pool(name="bld", bufs=1) as b_pool,
            tc.tile_pool(name="vwr", bufs=1) as v_pool,
            tc.tile_pool(name="evl", bufs=1) as e_pool,
            tc.tile_pool(name="zpsa", bufs=1, space="PSUM") as z_ps_a,
            tc.tile_pool(name="zpsb", bufs=1, space="PSUM") as z_ps_b,
            tc.tile_pool(name="yps", bufs=1, space="PSUM") as y_ps,
            tc.tile_pool(name="sps", bufs=1, space="PSUM") as s_ps,
        ):
            # ---- input DMAs: 6 total, small-first, all on SP so HWDGE +
            # DMA-engine order matches priority (weights last, per layer)
            t_cf = cst_pool.tile([128, CW], F32, tag="cf32")
            nc.sync.dma_start(t_cf[:], d_cf32[:, :])
            t_wh = [wh_pool.tile([128, 4096], F16, tag=f"wh{l}", name=f"wh{l}")
                    for l in range(NHID)]
            nc.sync.dma_start(t_wh[0][:], d_wh[:, 0:4096])
            t_r16 = cst_pool.tile([1, NHID * 16 * 128 + NS], F16, tag="r16")
            nc.sync.dma_start(t_r16[:], d_r16[:, :])
            for l in range(1, NHID):
                nc.sync.dma_start(t_wh[l][:], d_wh[:, l * 4096:(l + 1) * 4096])
            t_wo = wh_pool.tile([128, 128], F16, tag="wo")
            nc.sync.dma_start(t_wo[:], d_wh[:, NHID * 4096:NHID * 4096 + 128])

            t_g = cst_pool.tile([128, 64], F32R, tag="g")
            nc.gpsimd.dma_start(t_g[:], d_g[:, :])
            t_cst = t_cf[:, 0:64]
            ap_g = t_g[:, 0:64]
            t_xs = t_cf[:, 64:64 + NS]
            t_xb = t_cf[:, 64 + NS:64 + NS + NTILE * C]
            t_bh = t_r16[0:1, 0:NHID * 16 * 128]
            t_on = t_r16[0:1, NHID * 16 * 128:NHID * 16 * 128 + NS]

            ap_gw = t_cf[:, C_GW:C_GW + 1]
            ap_gb = t_cf[:, C_GB:C_GB + 1]
            ap_bo = t_cf[0:8, C_BO:C_BO + 1]

            # ---- eval prologue (runs during weight DMAs)
            # u = exp(gw*x + gb); t = x*M - bin; moments m_k = u * t^k.
            # The cubic+combine later folds into PE matmuls over these.
            EC = NTILE * C
            t_u = e_pool.tile([128, EC], F32R, tag="u")
            t_t = e_pool.tile([128, EC], F32, tag="t")
            t_zg = e_pool.tile([128, EC], F32, tag="zg")
            t_m1 = e_pool.tile([128, EC], F32R, tag="m1")
            t_m2 = e_pool.tile([128, EC], F32R, tag="m2")
            t_m3 = e_pool.tile([128, EC], F32R, tag="m3")
            nc.vector.tensor_scalar(t_zg[:], t_xb, ap_gw, ap_gb,
                                    ALU.mult, ALU.add)
            for hf in range(2):
                h0, h1 = hf * EC // 2, (hf + 1) * EC // 2
                nc.scalar.activation(t_u[:, h0:h1], t_zg[:, h0:h1], AT.Exp,
                                     bias=0.0, scale=1.0)
            for tt in range(NTILE):
                nc.vector.tensor_scalar(t_t[:, tt * C:(tt + 1) * C],
                                        t_xb[:, tt * C:(tt + 1) * C],
                                        float(M),
                                        t_cf[:, C_BIN + tt:C_BIN + tt + 1],
                                        ALU.mult, ALU.subtract)
            p_d = s_ps.tile([64, EC], F32, tag="aux", name="pd")
            i_dmm0 = nc.tensor.matmul(p_d[:, :], ap_g, t_u[:], start=True,
                                      stop=True)
            nc.gpsimd.tensor_tensor(t_m1[:], t_u[:], t_t[:], ALU.mult)
            nc.gpsimd.tensor_tensor(t_m2[:], t_m1[:], t_t[:], ALU.mult)
            nc.vector.tensor_tensor(t_m3[:], t_m2[:], t_t[:], ALU.mult)
            t_m32 = e_pool.tile([128, EC], F32R, tag="m32")
            t_m12 = e_pool.tile([128, EC], F32R, tag="m12")
            nc.vector.tensor_tensor(t_m32[:], t_m3[:], t_m2[:], ALU.subtract)
            nc.gpsimd.tensor_tensor(t_m12[:], t_m1[:], t_m2[:], ALU.subtract)

            # ---- layer 0 (Pool: zb = a*x + c; DVE: magic round; ACT: sin)
            t_zb = b_pool.tile([128, SLAB], F32, tag="zb")
            t_k = b_pool.tile([128, SLAB], F32, tag="k")
            t_v0 = b_pool.tile([128, SLAB], F32, tag="v0")
            t_h = [b_pool.tile([128, SLAB], F16, tag=f"h{l}", name=f"h{l}")
                   for l in range(NLAYERS)]
            for u in range(16):
                eng = nc.vector if u % 2 == 0 else nc.gpsimd
                eng.tensor_scalar(t_zb[:, ucol(u):ucol(u) + NS], t_xs,
                                  t_cf[:, C_A0 + u:C_A0 + u + 1],
                                  t_cf[:, C_C0 + u:C_C0 + u + 1],
                                  ALU.mult, ALU.add)
            for lo, hi in SPANS:
                sl = slice(lo, hi)
                nc.vector.tensor_scalar(t_k[:, sl], t_zb[:, sl], MAGIC, MAGIC,
                                        ALU.add, ALU.subtract)
                nc.vector.tensor_tensor(t_v0[:, sl], t_zb[:, sl], t_k[:, sl],
                                        ALU.subtract)
                nc.scalar.activation(t_h[0][:, sl], t_v0[:, sl], AT.Sin,
                                     bias=0.0, scale=TWO_PI)

            # ---- hidden layers: fp16 matmuls (+fp16 K=1 bias matmul), wrap, sin
            for l in range(1, NLAYERS):
                lw = l - 1
                p_z = (z_ps_a if l % 2 else z_ps_b).tile(
                    [128, SLAB], F32, tag="z", name=f"z{l}")
                for e in range(E):
                    for half in range(2):
                        u = e * 2 + half
                        out_sl = p_z[:, ucol(u):ucol(u) + NS]
                        for kc in range(2):
                            wc = ((e * 2 + kc) * 2 + half) * 128
                            nc.tensor.matmul(
                                out_sl, t_wh[lw][:, wc:wc + 128],
                                t_h[l - 1][:, ucol(e * 2 + kc):
                                            ucol(e * 2 + kc) + NS],
                                start=(kc == 0), stop=False)
                        bc = (lw * 16 + u) * 128
                        nc.tensor.matmul(out_sl, t_r16[0:1, bc:bc + 128],
                                         t_on, start=False, stop=True)
                t_v = v_pool.tile([128, SLAB], F32, tag="v", name=f"v{l}",
                                  bufs=2)
                for lo, hi in SPANS:
                    sl = slice(lo, hi)
                    if wrap_twice:
                        t_w2 = v_pool.tile([128, SLAB], F32, tag="w2",
                                           name=f"w2_{l}{lo}", bufs=2)
                        nc.vector._custom_dve(ADD_RANGE_WRAP,
                                              out=t_w2[:, sl],
                                              in0=p_z[:, sl], s0=0.0,
                                              s1=1.0, imm2=2.0)
                        nc.vector._custom_dve(ADD_RANGE_WRAP, out=t_v[:, sl],
                                              in0=t_w2[:, sl], s0=0.0,
                                              s1=0.5, imm2=1.0)
                    else:
                        nc.vector._custom_dve(ADD_RANGE_WRAP, out=t_v[:, sl],
                                              in0=p_z[:, sl], s0=0.0,
                                              s1=0.5, imm2=1.0)
                    nc.scalar.activation(t_h[l][:, sl], t_v[:, sl], AT.Sin,
                                         bias=0.0, scale=TWO_PI)

            # ---- output layer -> table [8, NS]
            p_y = y_ps.tile([8, NS], F32, tag="y")
            for e in range(E):
                for kc in range(2):
                    blk = (e * 2 + kc) * 8
                    i_ymm = nc.tensor.matmul(
                        p_y[:, :], t_wo[:, blk:blk + 8],
                        t_h[NLAYERS - 1][:, ucol(e * 2 + kc):
                                         ucol(e * 2 + kc) + NS],
                        start=(e == 0 and kc == 0), stop=(e == 7 and kc == 1),
                        skip_group_check=True)
            # ---- Catmull-Rom coefficients in table layout (free-dim shifts),
            # written into a stride-spread tile so ONE 3-dim DMA lands them as
            # per-partition scalars: cf[e, bl*68 + tt*4 + X] -> coef[e*16+bl,
            # tt*4+X], X in {A,B,C,D}.
            SPR = 68
            t_cf = b_pool.tile([8, 16 * SPR], F32, tag="cf")

            t_tab = b_pool.tile([8, NS], F32, tag="tab")
            i_tab = nc.vector.tensor_scalar(t_tab[:], p_y[:], ap_bo, 0.0,
                                            ALU.add, ALU.add)

            def pk(k):
                return bass.AP(t_tab.tensor, t_tab[0:1, k:k + 1].offset,
                               [[NS, 8], [1, BPC]])

            def spread(X):
                return bass.AP(t_cf.tensor, t_cf[0:1, X:X + 1].offset,
                               [[16 * SPR, 8], [4, 4], [SPR, 16]])

            t_t2 = b_pool.tile([8, BPC], F32, tag="t2")
            i_t1 = nc.vector.tensor_tensor(spread(1), pk(1), pk(2),
                                           ALU.subtract)          # t1
            i_t2 = nc.gpsimd.tensor_tensor(t_t2[:], pk(3), pk(0),
                                           ALU.subtract)
            i_A = nc.vector.scalar_tensor_tensor(spread(0), spread(1), 3.0,
                                                 t_t2[:], ALU.mult,
                                                 ALU.add)          # A'
            i_C = nc.gpsimd.tensor_tensor(spread(2), pk(2), pk(0),
                                          ALU.subtract)            # C'
            i_D = nc.gpsimd.tensor_scalar(spread(3), pk(1), 1.0, 0.0,
                                          ALU.mult, ALU.add)       # D
            t_coef = e_pool.tile([128, 16], F32, tag="coef")
            srcc = bass.AP(t_cf.tensor, t_cf[0:1, 0:1].offset,
                           [[16 * SPR, 8], [SPR, 16], [1, 16]])
            i_cdma = nc.sync.dma_start(t_coef[:], srcc)
            # raw-AP operands escape tile's subtile dependency tracking;
            # wire the hazards explicitly.
            for rd in (i_t1, i_t2, i_C, i_D):
                add_dep_helper(rd.ins, i_tab.ins, reason="pk reads tab")
            add_dep_helper(i_A.ins, i_t1.ins, reason="A reads spread t1")
            for wr in (i_A, i_t1, i_C, i_D):
                add_dep_helper(i_cdma.ins, wr.ins, reason="coef dma reads cf")
            t_cA = [t_coef[:, tt * 4 + 0:tt * 4 + 1] for tt in range(NTILE)]
            t_cB = [t_coef[:, tt * 4 + 1:tt * 4 + 2] for tt in range(NTILE)]
            t_cC = [t_coef[:, tt * 4 + 2:tt * 4 + 3] for tt in range(NTILE)]
            t_cD = [t_coef[:, tt * 4 + 3:tt * 4 + 4] for tt in range(NTILE)]

            # ---- fold cubic + gate-weighted combine into PE matmuls:
            # lhsT Gall[(tt,X)] = G * coef_X(tile tt) per partition; then
            # N = sum_X Gall(X).T @ m_X accumulated in PSUM; D = G.T @ u.
            t_gall = e_pool.tile([128, 256], F32R, tag="gall")
            GSCL = [0.5, -1.0, 0.5, 1.0]
            i_gall = []
            for X in range(4):
                for tt in range(NTILE):
                    eng = nc.vector if (X * 4 + tt) % 2 == 0 else nc.gpsimd
                    i_gall.append(eng.tensor_scalar(
                        t_gall[:, X * 64 + tt * 16:X * 64 + (tt + 1) * 16],
                        t_g[:, tt * 16:(tt + 1) * 16],
                        t_coef[:, tt * 4 + X:tt * 4 + X + 1],
                        GSCL[X], ALU.mult, ALU.mult))
            p_n = s_ps.tile([64, EC], F32, tag="aux", name="pn")
            mX = [t_m32, t_m2, t_m12, t_u]
            for X in range(4):
                nc.tensor.matmul(p_n[:, :], t_gall[:, X * 64:(X + 1) * 64],
                                 mX[X][:], start=(X == 0), stop=(X == 3))
            t_rcp = e_pool.tile([64, EC], F32, tag="rcp")
            t_out = e_pool.tile([64, EC], F32, tag="out")
            nc.vector.reciprocal(t_rcp[:], p_d[:])
            nc.vector.tensor_tensor(t_out[:], p_n[:], t_rcp[:], ALU.mult)
            nc.sync.dma_start(d_out[:, :], t_out[:])

    nc.compile()
    return nc


def _host_pack(x, gate_w, gate_b, w0, b0, wh, bh, wo, bo, C):
    """Host: range-shard + bin points, pack weights/consts. Index math only."""
    xf = x.reshape(-1)
    NP = xf.size
    gbin = np.clip((xf.astype(np.float64) * M).astype(np.int64), 0, M - 1)
    core = gbin >> 6
    binlo = gbin & (BPC - 1)

    # slot assignment per (core, local bin)
    counts = np.zeros((NCORES, BPC), np.int64)
    slot = np.empty(NP, np.int64)
    for i in range(NP):
        c, b = core[i], binlo[i]
        slot[i] = counts[c, b]
        counts[c, b] += 1
    maxc = int(counts.max())
    if maxc > C:
        return None, None, maxc  # caller grows C and recompiles

    in_maps = []
    placement = (core, binlo, slot)

    # fp16 weight packs (shared across cores)
    whp = np.zeros((128, NHID * 4096), np.float16)
    for l in range(NHID):
        for e in range(E):
            for kc in range(2):
                for mc in range(2):
                    colbase = l * 4096 + ((e * 2 + kc) * 2 + mc) * 128
                    blk = (SC * wh[l, e, mc * 128:(mc + 1) * 128,
                                   kc * 128:(kc + 1) * 128]).T
                    whp[:, colbase:colbase + 128] = blk.astype(np.float16)
    bhp = np.zeros((1, NHID * 16 * 128), np.float16)
    for l in range(NHID):
        for u in range(16):
            e, half = divmod(u, 2)
            bhp[0, (l * 16 + u) * 128:(l * 16 + u + 1) * 128] = \
                (SC * bh[l, e, half * 128:(half + 1) * 128]).astype(np.float16)
    wop = np.zeros((128, 128), np.float16)
    for e in range(E):
        for kc in range(2):
            wop[:, (e * 2 + kc) * 8 + e] = \
                wo[e, 0, kc * 128:(kc + 1) * 128].astype(np.float16)
    r16 = np.zeros((1, NHID * 16 * 128 + NS), np.float16)
    r16[0, :NHID * 16 * 128] = bhp[0]
    r16[0, NHID * 16 * 128:] = 1.0
    whm = np.zeros((128, NHID * 4096 + 128), np.float16)
    whm[:, :NHID * 4096] = whp
    whm[:, NHID * 4096:] = wop
    gp = np.zeros((128, 64), np.float32)
    for e in range(E):
        for bl in range(16):
            for tt in range(NTILE):
                gp[e * 16 + bl, tt * 16 + bl] = 1.0

    cst_base = np.zeros((128, 64), np.float32)
    for u in range(16):
        e, half = divmod(u, 2)
        cst_base[:, C_A0 + u] = SC * w0[e, half * 128:(half + 1) * 128, 0]
        cst_base[:, C_C0 + u] = SC * b0[e, half * 128:(half + 1) * 128]
    rows_e = np.repeat(np.arange(E), 16)          # row p=e*16+bl -> e
    rows_bl = np.tile(np.arange(16), E)
    cst_base[:, C_GW] = gate_w[rows_e, 0]
    cst_base[:, C_GB] = gate_b[rows_e]
    cst_base[0:8, C_BO] = bo[:, 0]

    for j in range(NCORES):
        cst = cst_base.copy()
        for tt in range(NTILE):
            cst[:, C_BIN + tt] = j * BPC + tt * 16 + rows_bl
        # binned x, replicated over the 8 expert rows of each bin
        xbj = np.empty((128, NTILE * C), np.float32)
        for tt in range(NTILE):
            for bl in range(16):
                gb = j * BPC + tt * 16 + bl
                fill = np.full(C, (gb + 0.5) / M, np.float32)
                sel = (core == j) & (binlo == tt * 16 + bl)
                vals = xf[sel]
                fill[:vals.size] = vals
                for e in range(E):
                    xbj[e * 16 + bl, tt * C:(tt + 1) * C] = fill
        # build sample positions
        gi = j * BPC - 1 + np.minimum(np.arange(NS), BPC + 2)
        xsj = np.broadcast_to((gi / M).astype(np.float32), (128, NS))
        cf32 = np.empty((128, 64 + NS + NTILE * C), np.float32)
        cf32[:, 0:64] = cst
        cf32[:, 64:64 + NS] = xsj
        cf32[:, 64 + NS:] = xbj
        in_maps.append({"cf32": cf32, "g": gp, "r16": r16, "wh": whm})
    return in_maps, placement, maxc


def kernel(x, gate_w, gate_b, w0, b0, wh, bh, wo, bo):
    x = np.asarray(x, dtype=np.float32)
    gate_w = np.asarray(gate_w, dtype=np.float32)
    gate_b = np.asarray(gate_b, dtype=np.float32)
    w0 = np.asarray(w0, dtype=np.float32)
    b0 = np.asarray(b0, dtype=np.float32)
    wh = np.asarray(wh, dtype=np.float32)
    bh = np.asarray(bh, dtype=np.float32)
    wo = np.asarray(wo, dtype=np.float32)
    bo = np.asarray(bo, dtype=np.float32)

    # hidden pre-activation range (turns) over the union of build grids
    gi = np.arange(-1, M + NS - BPC + 2, dtype=np.float64) / M
    xs = gi.astype(np.float32)
    a = (SC * w0[:, :, 0]).astype(np.float32)
    c = (SC * b0).astype(np.float32)
    zb = a[:, :, None] * xs[None, None, :] + c[:, :, None]
    h = np.sin(TWO_PI * (zb - np.round(zb))).astype(np.float32)
    hid_bound = 0.0
    for l in range(NHID):
        z = (np.einsum('egh,eht->egt', (SC * wh[l]).astype(np.float32), h)
             + (SC * bh[l]).astype(np.float32)[:, :, None])
        hid_bound = max(hid_bound, float(np.abs(z).max()))
        h = np.sin(TWO_PI * (z - np.round(z))).astype(np.float32)
    hid_bound *= 1.02
    assert hid_bound < 2.90, f"hidden range {hid_bound} needs >2 wraps"
    wrap_twice = bool(hid_bound >= 1.45)

    C = 112
    while True:
        in_maps, placement, maxc = _host_pack(
            x, gate_w, gate_b, w0, b0, wh, bh, wo, bo, C)
        if in_maps is not None:
            break
        C = ((maxc + 15) // 16) * 16

    key = (wrap_twice, C)
    if key not in _BUILD_CACHE:
        _BUILD_CACHE[key] = _build(wrap_twice, C)
    nc = _BUILD_CACHE[key]

    global LAST_RESULT
    LAST_RESULT = run_bass_kernel_spmd(nc, in_maps, list(range(NCORES)))
    res = LAST_RESULT.results

    core, binlo, slot = placement
    out = np.empty(x.size, np.float32)
    for j in range(NCORES):
        oj = res[j]["out"]            # [64, NTILE*C]
        sel = np.nonzero(core == j)[0]
        out[sel] = oj[binlo[sel], (binlo[sel] >> 4) * C + slot[sel]]
    return out.reshape(B, N, 1).astype(np.float32)
